# revision 33
# baseline (speedup 1.0000x reference)
"""EntropicLayer (GCN conv + entropy gradient) on 8 trn2 NeuronCores — v3.

3-launch SPMD, node-partitioned (dst-sharded edges), device does all
O(E*D) message passing and O(N*D^2) transforms; host does per-node
scalar glue between launches (dinv/bias/q/E/softmax/c) and layout.

  L1: gather xs=x*dinv_src (bf16) per dst-owned edge; per dst tile
      aggT[feat,node] = sum_blk g_blk^T @ sel_blk (unweighted one-hot sel,
      one broadcast is_equal per batch) + xsT_own tile add (self-loops);
      z~T = W^T @ aggT; single wide store.
      Host: z = dinv*z~ + b; q = ||z||^2; segq; E prep.
  L2: gather z (bf16) per edge; CT[feat,node] aggregation; single store.
      Host: C; zC; E = segq + indeg*q - 2 zC; softmax -> c (fp64);
      prune |c| <= tau; f0 = 1 + 2w(c*indeg + A); pack active tiles.
  L3: only tiles containing affected nodes (~dozens): gather packed
      z/C rows; weighted sel (-2w*c); psB aggregation node-major;
      o = z*f0 + psB; host scatters active tiles into out = z.
All matmuls bf16 (4x fp32 PE rate), fp32 PSUM accumulation.
"""

import math
import numpy as np
import ml_dtypes

import concourse.bass as bass
import concourse.bacc as bacc
import concourse.mybir as mybir
import concourse.tile as tile

P = 128
D = 128
F32 = mybir.dt.float32
BF16 = mybir.dt.bfloat16
I16 = mybir.dt.int16
AL = mybir.AluOpType
ACTF = mybir.ActivationFunctionType
AX = mybir.AxisListType
BF = ml_dtypes.bfloat16

N_NODES = 50000
N_CORES = 8

import os as _os
GCH = int(_os.environ.get("KGCH", "512"))
DDS = int(_os.environ.get("KDDS", "65536"))
TB = int(_os.environ.get("KTB", "4"))
KCW = int(_os.environ.get("KCW", "32"))
KNOID = _os.environ.get("KNOID", "0") == "1"
C_TAU = 1e-7  # |c| threshold for L3 edge pruning
L3_EDGE_CAP = 32768  # adaptive tau fallback cap


def cdiv(a, b):
    return (a + b - 1) // b


def bf(a):
    return np.ascontiguousarray(np.asarray(a).astype(BF))


# ----------------------------------------------------------------------------
# Host-side layout prep
# ----------------------------------------------------------------------------

class PassLayout:
    """Per-pass slot/block layout, uniform across cores.

    Aggregation windows are CW node-columns wide (CW <= 128): each block is
    [128 slots x CW cols], shrinking one-hot sel area by 128/CW while PSUM
    windows of 4 CW-tiles share one [128,128] accumulation tile.
    """

    def __init__(self, N, n_cores, TB=TB, cw=KCW):
        self.N = N
        self.n_cores = n_cores
        self.R = N // n_cores
        self.NT = cdiv(self.R, P)
        self.RP = self.NT * P
        self.CW = cw
        self.WPT = P // cw  # windows per 128-tile
        self.NW = self.NT * self.WPT
        self.HALF = min(32768, (N + 1) // 2)
        assert self.HALF <= 32768 and N - self.HALF <= 32768
        self.TB = TB
        self.batches = []
        t = 0
        while t < self.NT:
            ntk = min(TB, self.NT - t)
            self.batches.append((t, ntk))
            t += ntk

    def build(self, gidx, cnode, min_blocks=1, gc=None):
        """gidx: gather-node per edge; cnode: scatter-node per edge.

        gc: optional [N] grid-column map (node -> column in its owner's
        [P, RP] output grid), from host bin-packing. Default: node order.
        """
        N, n_cores, R, NW, HALF = self.N, self.n_cores, self.R, self.NW, self.HALF
        CW, WPT = self.CW, self.WPT
        gidx = np.asarray(gidx, dtype=np.int64)
        cnode = np.asarray(cnode, dtype=np.int64)
        owner = cnode // R
        if gc is None:
            loc = cnode - owner * R
            wl = loc // CW  # window index
            col = loc % CW
        else:
            gcp = gc[cnode]
            wl = (gcp // P) * WPT + (gcp % P) // CW
            col = gcp % CW
        h = (gidx >= HALF).astype(np.int64)

        key = (owner * NW + wl) * 2 + h
        counts = np.bincount(key, minlength=n_cores * NW * 2).reshape(n_cores, NW, 2)
        BA_w = np.maximum(min_blocks, (counts[:, :, 0].max(axis=0) + P - 1) // P).astype(np.int64)
        BB_w = np.maximum(min_blocks, (counts[:, :, 1].max(axis=0) + P - 1) // P).astype(np.int64)
        self.BA_w, self.BB_w = BA_w, BB_w
        astart = np.zeros(NW + 1, dtype=np.int64)
        astart[1:] = np.cumsum(BA_w * P)
        bstart = np.zeros(NW + 1, dtype=np.int64)
        bstart[1:] = np.cumsum(BB_w * P)
        self.astart, self.bstart = astart, bstart
        self.SA = int(astart[-1])
        self.SB = int(bstart[-1])
        ABLK0 = np.zeros(NW, dtype=np.int64)
        BBLK0 = np.zeros(NW, dtype=np.int64)
        base_blk = []
        acc = 0
        maxb = 0
        for (t0, ntk) in self.batches:
            base_blk.append(acc)
            a = acc
            for j in range(ntk * WPT):
                ABLK0[t0 * WPT + j] = a
                a += int(BA_w[t0 * WPT + j])
            for j in range(ntk * WPT):
                BBLK0[t0 * WPT + j] = a
                a += int(BB_w[t0 * WPT + j])
            maxb = max(maxb, a - acc)
            acc = a
        self.ABLK0, self.BBLK0 = ABLK0, BBLK0
        self.base_blk = base_blk + [acc]
        self.NBLK = max(acc, 1)
        self.MAXB = max(maxb, 1)
        self.SA = max(self.SA, 16)
        self.SB = max(self.SB, 16)

        order = np.argsort(key, kind="stable")
        ks = key[order]
        group_start = np.zeros(n_cores * NW * 2, dtype=np.int64)
        cnt_flat = np.bincount(key, minlength=n_cores * NW * 2)
        group_start[1:] = np.cumsum(cnt_flat)[:-1]
        pos = np.arange(len(order)) - group_start[ks]

        og, oc, ow, oh, ocol = (gidx[order], owner[order], wl[order], h[order], col[order])
        oeid = order  # original edge ordinal per sorted slot

        cores = []
        for c in range(n_cores):
            m = oc == c
            cg, cww, ch, ccol, cpos, ceid = og[m], ow[m], oh[m], ocol[m], pos[m], oeid[m]

            idxA = np.zeros(self.SA, dtype=np.int16)
            idxB = np.zeros(self.SB, dtype=np.int16)
            colid = np.full((self.NBLK, P), -1.0, dtype=np.float32)
            eord = np.full((self.NBLK, P), -1, dtype=np.int64)

            for half, (idxarr, wstart, wblk0, off) in enumerate(
                [(idxA, astart, ABLK0, 0), (idxB, bstart, BBLK0, HALF)]
            ):
                mm = ch == half
                ww, ppos, gg, ccc, ee = cww[mm], cpos[mm], cg[mm], ccol[mm], ceid[mm]
                stream = wstart[ww] + ppos
                idxarr[stream] = (gg - off).astype(np.int16)
                i = ppos // P
                p = ppos % P
                blk = wblk0[ww] + i
                colid[blk, p] = ccc.astype(np.float32)
                eord[blk, p] = ee

            cores.append(
                dict(
                    idxA=self._wrap(idxA),
                    idxB=self._wrap(idxB),
                    colid=np.ascontiguousarray(colid.T),  # [P, NBLK] f32
                    eord=eord,  # [NBLK, P] edge ordinal or -1
                )
            )
        self.cores = cores
        return self

    @staticmethod
    def _wrap(flat):
        S = len(flat)
        assert S % 16 == 0
        arr = flat.reshape(S // 16, 16).T  # [16, S/16]
        return np.ascontiguousarray(np.tile(arr, (8, 1)))  # [128, S/16]

    def permute_edges(self, core, edge_vec, padval, dtype=np.float32):
        """[P, NBLK] array with edge_vec[eord] (padval on pads)."""
        eord = self.cores[core]["eord"]
        out = np.full(eord.shape, padval, dtype=np.float64)
        m = eord >= 0
        out[m] = edge_vec[eord[m]]
        return np.ascontiguousarray(out.T.astype(dtype))  # [P, NBLK]


def _pack_core(cntA, cntB, capA, capB, CW):
    """Greedy-pack one core's nodes into windows under (A, B) capacities.
    Returns gc [R] grid column per local node, or None if infeasible."""
    NW = len(capA)
    R = len(cntA)
    order = np.argsort(-(cntA + cntB), kind="stable")
    sumA = np.zeros(NW, dtype=np.int64)
    sumB = np.zeros(NW, dtype=np.int64)
    nn = np.zeros(NW, dtype=np.int64)
    assign_w = np.empty(R, dtype=np.int64)
    assign_j = np.empty(R, dtype=np.int64)
    for v in order:
        a, b = int(cntA[v]), int(cntB[v])
        rA = capA - sumA - a
        rB = capB - sumB - b
        feas = (rA >= 0) & (rB >= 0) & (nn < CW)
        if not feas.any():
            return None
        score = np.where(feas, rA + rB, -1)
        w = int(np.argmax(score))
        assign_w[v] = w
        assign_j[v] = nn[w]
        sumA[w] += a
        sumB[w] += b
        nn[w] += 1
    WPT = P // CW
    return (assign_w // WPT) * P + (assign_w % WPT) * CW + assign_j


def _pack_all(src, dst, N, n_cores, lay):
    """Shared bimodal window-capacity profile + per-core packing."""
    R = N // n_cores
    NW, CW = lay.NW, lay.CW
    isA = (src < lay.HALF).astype(np.int64)
    cA = np.bincount(dst, weights=isA, minlength=N).astype(np.int64)
    cB = np.bincount(dst, weights=1 - isA, minlength=N).astype(np.int64)
    maxA = max(int(cA[c * R:(c + 1) * R].sum()) for c in range(n_cores))
    maxB = max(int(cB[c * R:(c + 1) * R].sum()) for c in range(n_cores))
    for margin in (4, 8, 16, 32):
        n2A = min(NW, max(0, cdiv(maxA + margin * 128 - 124 * NW, 128)))
        n2B = min(NW, max(0, cdiv(maxB + margin * 128 - 124 * NW, 128)))
        capA = np.full(NW, 124, dtype=np.int64)
        capA[:n2A] = 252
        capB = np.full(NW, 124, dtype=np.int64)
        capB[NW - n2B:] = 252  # stagger A/B heavy windows
        if capA.sum() < maxA or capB.sum() < maxB:
            continue
        gc = np.empty(N, dtype=np.int64)
        ok = True
        for c in range(n_cores):
            lo = c * R
            g = _pack_core(cA[lo:lo + R], cB[lo:lo + R], capA, capB, CW)
            if g is None:
                ok = False
                break
            gc[lo:lo + R] = g
        if ok:
            return gc
    return None


def host_prep(edge_index, N, n_cores):
    src = np.asarray(edge_index[0], dtype=np.int64)
    dst = np.asarray(edge_index[1], dtype=np.int64)
    E = len(src)

    deg = np.bincount(dst, minlength=N).astype(np.float64) + 1.0
    dinv = deg ** -0.5
    indeg = np.bincount(dst, minlength=N).astype(np.float64)

    R = N // n_cores
    lay = PassLayout(N, n_cores)
    gc = _pack_all(src, dst, N, n_cores, lay) if _os.environ.get(
        "KPACK", "0") == "1" else None
    L12 = lay.build(src, dst, gc=gc)
    return dict(src=src, dst=dst, dinv=dinv, indeg=indeg,
                L12=L12, gc=gc, R=R, N=N, E=E)


# ----------------------------------------------------------------------------
# Device builders
# ----------------------------------------------------------------------------

IOTA_BF = np.tile(np.arange(P, dtype=np.float32), (P, 1)).astype(BF)


def _gather_batches(nc, lay, gpool, ipool, x_dram, idxA, idxB):
    """Yield (k, t0, ntk, gtile) with both A and B regions gathered (bf16).

    idx slices are DMA-loaded per batch (pipelined) so the first gather
    starts ~5us earlier than with monolithic idx loads."""
    HALF = lay.HALF
    N = lay.N
    aofs = 0
    bofs = 0
    qn = 0
    for k, (t0, ntk) in enumerate(lay.batches):
        g = gpool.tile([P, lay.MAXB, D], BF16, tag="gat")
        w0, w1 = t0 * lay.WPT, (t0 + ntk) * lay.WPT
        nAblk = int(sum(lay.BA_w[w0:w1]))
        nBblk = int(sum(lay.BB_w[w0:w1]))
        nA = nAblk * P
        nB = nBblk * P
        idx_t = ipool.tile([P, (nA + nB) // 16], I16, tag="idx")
        nc.sync.dma_start(idx_t[:, 0 : nA // 16],
                          idxA[:, aofs : aofs + nA // 16])
        nc.sync.dma_start(idx_t[:, nA // 16 : (nA + nB) // 16],
                          idxB[:, bofs : bofs + nB // 16])
        for off in range(0, nA, GCH):
            n = min(GCH, nA - off)
            nc.gpsimd.dma_gather(
                g[:, off // P : (off + n) // P, :],
                x_dram[0:HALF, :],
                idx_t[:, off // 16 : (off + n) // 16],
                n, n, D, elem_step=D, queue_num=qn,
            )
            qn = 1 - qn
        for off in range(0, nB, GCH):
            n = min(GCH, nB - off)
            nc.gpsimd.dma_gather(
                g[:, nAblk + off // P : nAblk + (off + n) // P, :],
                x_dram[HALF:N, :],
                idx_t[:, (nA + off) // 16 : (nA + off + n) // 16],
                n, n, D, elem_step=D, queue_num=qn,
            )
            qn = 1 - qn
        aofs += nA // 16
        bofs += nB // 16
        yield k, t0, ntk, g


def _sel_batch(nc, spool, colid_t, iota_t, blk0, NB, CW):
    """Unweighted one-hot sel (bf16) for NB consecutive blocks, one DVE op."""
    sel = spool.tile([P, NB * CW], BF16, tag="sel")
    s3 = sel[:].rearrange("p (b f) -> p b f", f=CW)
    iap = iota_t[:, 0:CW]
    iota3 = bass.AP(iap.tensor, iap.offset, [iap.ap[0], [0, NB], iap.ap[1]])
    nc.vector.tensor_tensor(
        out=s3,
        in0=colid_t[:, blk0 : blk0 + NB].to_broadcast([P, NB, CW]),
        in1=iota3,
        op=AL.is_equal,
    )
    return sel


def _wblocks(lay, t, k):
    """Per 128-tile t in batch k: per-window lists of (buf, blk)."""
    base = lay.base_blk[k]
    out = []
    for wi in range(lay.WPT):
        w = t * lay.WPT + wi
        wb = []
        for i in range(int(lay.BA_w[w])):
            wb.append((int(lay.ABLK0[w]) - base + i, int(lay.ABLK0[w]) + i))
        for i in range(int(lay.BB_w[w])):
            wb.append((int(lay.BBLK0[w]) - base + i, int(lay.BBLK0[w]) + i))
        out.append(wb)
    return out


def build_L1(lay, N):
    nc = bacc.Bacc("TRN2", target_bir_lowering=False, debug=False,
                   dynamic_dma_scratch_size=DDS, num_swdge_queues=2)
    NT, RP, NBLK = lay.NT, lay.RP, lay.NBLK
    SA16, SB16 = lay.SA // 16, lay.SB // 16

    x = nc.dram_tensor("x", [N, D], BF16, kind="ExternalInput")
    Wm = nc.dram_tensor("W", [D, D], BF16, kind="ExternalInput")
    idxA = nc.dram_tensor("idxA", [P, SA16], I16, kind="ExternalInput")
    idxB = nc.dram_tensor("idxB", [P, SB16], I16, kind="ExternalInput")
    colid = nc.dram_tensor("colid", [P, NBLK], BF16, kind="ExternalInput")
    xsT = nc.dram_tensor("xsT", [P, RP], BF16, kind="ExternalInput")
    iota_c = nc.inline_tensor(IOTA_BF, "iota_c")
    ident_c = nc.inline_tensor(np.eye(P, dtype=np.float32).astype(BF), "ident_c")

    zT_out = nc.dram_tensor("zT", [P, RP], F32, kind="ExternalOutput")
    CW = lay.CW

    with tile.TileContext(nc) as tc:
        with (
            tc.tile_pool(name="const", bufs=1) as cpool,
            tc.tile_pool(name="gat", bufs=2) as gpool,
            tc.tile_pool(name="idx", bufs=3) as ipool,
            tc.tile_pool(name="sel", bufs=2) as spool,
            tc.tile_pool(name="work", bufs=3) as wpool,
            tc.tile_pool(name="acc", bufs=1) as apool,
            tc.tile_pool(name="psA", bufs=2, space="PSUM") as psA_pool,
            tc.tile_pool(name="psZ", bufs=2, space="PSUM") as psZ_pool,
        ):
            # const loads on DVE/Act HWDGE queues so SP starts with the
            # batch-0 idx slices (first gather ~1us instead of ~9us)
            colid_t = cpool.tile([P, NBLK], BF16)
            nc.scalar.dma_start(colid_t[:], colid[:])
            iota_t = cpool.tile([P, P], BF16)
            nc.scalar.dma_start(iota_t[:], iota_c[:])
            ident_t = cpool.tile([P, P], BF16)
            nc.scalar.dma_start(ident_t[:], ident_c[:])
            W_t = cpool.tile([P, P], BF16)
            nc.scalar.dma_start(W_t[:], Wm[:])
            xsT_t = cpool.tile([P, RP], BF16)
            nc.scalar.dma_start(xsT_t[:], xsT[:])
            zrow = apool.tile([P, RP], F32)

            for k, t0, ntk, g in _gather_batches(nc, lay, gpool, ipool, x,
                                                 idxA, idxB):
                base = lay.base_blk[k]
                NBk = lay.base_blk[k + 1] - base
                sel = _sel_batch(nc, spool, colid_t, iota_t, base, NBk, CW)
                for j in range(ntk):
                    t = t0 + j
                    psA = psA_pool.tile([P, P], F32, tag="agg")
                    for wi, blocks in enumerate(_wblocks(lay, t, k)):
                        win = psA[:, wi * CW : (wi + 1) * CW]
                        c0 = t * P + wi * CW
                        nb = len(blocks)
                        if not KNOID:
                            # self-loop row add via identity matmul (starts group)
                            nc.tensor.matmul(
                                win, lhsT=ident_t[:], rhs=xsT_t[:, c0 : c0 + CW],
                                start=True, stop=False,
                            )
                        for bi, (buf, blk) in enumerate(blocks):
                            nc.tensor.matmul(
                                win, lhsT=g[:, buf, :],
                                rhs=sel[:, buf * CW : (buf + 1) * CW],
                                start=(KNOID and bi == 0), stop=(bi == nb - 1),
                            )
                    aggT_s = wpool.tile([P, P], BF16, tag="aggTs")
                    if KNOID:
                        nc.vector.tensor_tensor(
                            out=aggT_s[:], in0=psA[:],
                            in1=xsT_t[:, t * P : (t + 1) * P], op=AL.add,
                        )
                    else:
                        nc.any.tensor_copy(out=aggT_s[:], in_=psA[:])
                    psZ = psZ_pool.tile([P, P], F32, tag="zT")
                    nc.tensor.matmul(psZ[:], lhsT=W_t[:], rhs=aggT_s[:],
                                     start=True, stop=True)
                    nc.any.tensor_copy(out=zrow[:, t * P : (t + 1) * P], in_=psZ[:])
                nc.sync.dma_start(
                    zT_out[:, t0 * P : (t0 + ntk) * P],
                    zrow[:, t0 * P : (t0 + ntk) * P])
    nc.compile()
    return nc


def build_L2(lay, N):
    nc = bacc.Bacc("TRN2", target_bir_lowering=False, debug=False,
                   dynamic_dma_scratch_size=DDS, num_swdge_queues=2)
    NT, RP, NBLK = lay.NT, lay.RP, lay.NBLK
    SA16, SB16 = lay.SA // 16, lay.SB // 16

    z = nc.dram_tensor("z", [N, D], BF16, kind="ExternalInput")
    idxA = nc.dram_tensor("idxA", [P, SA16], I16, kind="ExternalInput")
    idxB = nc.dram_tensor("idxB", [P, SB16], I16, kind="ExternalInput")
    colid = nc.dram_tensor("colid", [P, NBLK], BF16, kind="ExternalInput")
    iota_c = nc.inline_tensor(IOTA_BF, "iota_c")

    CT_out = nc.dram_tensor("CT", [P, RP], F32, kind="ExternalOutput")

    with tile.TileContext(nc) as tc:
        with (
            tc.tile_pool(name="const", bufs=1) as cpool,
            tc.tile_pool(name="gat", bufs=2) as gpool,
            tc.tile_pool(name="idx", bufs=3) as ipool,
            tc.tile_pool(name="sel", bufs=2) as spool,
            tc.tile_pool(name="acc", bufs=1) as apool,
            tc.tile_pool(name="psC", bufs=2, space="PSUM") as psC_pool,
        ):
            colid_t = cpool.tile([P, NBLK], BF16)
            nc.scalar.dma_start(colid_t[:], colid[:])
            iota_t = cpool.tile([P, P], BF16)
            nc.scalar.dma_start(iota_t[:], iota_c[:])
            Crow = apool.tile([P, RP], F32)
            CW = lay.CW

            for k, t0, ntk, g in _gather_batches(nc, lay, gpool, ipool, z,
                                                 idxA, idxB):
                base = lay.base_blk[k]
                NBk = lay.base_blk[k + 1] - base
                sel = _sel_batch(nc, spool, colid_t, iota_t, base, NBk, CW)
                for j in range(ntk):
                    t = t0 + j
                    psC = psC_pool.tile([P, P], F32, tag="C")
                    for wi, blocks in enumerate(_wblocks(lay, t, k)):
                        win = psC[:, wi * CW : (wi + 1) * CW]
                        nb = len(blocks)
                        for bi, (buf, blk) in enumerate(blocks):
                            nc.tensor.matmul(
                                win, lhsT=g[:, buf, :],
                                rhs=sel[:, buf * CW : (buf + 1) * CW],
                                start=(bi == 0), stop=(bi == nb - 1),
                            )
                    nc.any.tensor_copy(out=Crow[:, t * P : (t + 1) * P], in_=psC[:])
                nc.sync.dma_start(
                    CT_out[:, t0 * P : (t0 + ntk) * P],
                    Crow[:, t0 * P : (t0 + ntk) * P])
    nc.compile()
    return nc


def build_L3(NT3, NB3_t, M3, S3):
    """Active-tile combine: o = z*f0 + sum_blk sel_w^T @ g  (node-major).

    NT3: active tiles per core (uniform); NB3_t[i]: blocks for slot i;
    M3: packed gather source rows; S3: total gather slots (mult of 16).
    """
    nc = bacc.Bacc("TRN2", target_bir_lowering=False, debug=False,
                   dynamic_dma_scratch_size=16384, num_swdge_queues=1)
    NBLK3 = int(sum(NB3_t))
    S16 = S3 // 16

    zc = nc.dram_tensor("zc", [M3, D], BF16, kind="ExternalInput")
    idx3 = nc.dram_tensor("idx3", [P, S16], I16, kind="ExternalInput")
    colid = nc.dram_tensor("colid", [P, max(NBLK3, 1)], F32, kind="ExternalInput")
    csl = nc.dram_tensor("csl", [P, max(NBLK3, 1)], F32, kind="ExternalInput")
    zog = nc.dram_tensor("zog", [P, NT3 * D], F32, kind="ExternalInput")
    f0g = nc.dram_tensor("f0g", [P, NT3], F32, kind="ExternalInput")
    iota_c = nc.inline_tensor(
        np.tile(np.arange(P, dtype=np.float32), (P, 1)), "iota_c")

    out = nc.dram_tensor("out", [P, NT3 * D], F32, kind="ExternalOutput")

    with tile.TileContext(nc) as tc:
        with (
            tc.tile_pool(name="const", bufs=1) as cpool,
            tc.tile_pool(name="sel", bufs=2) as spool,
            tc.tile_pool(name="work", bufs=3) as wpool,
            tc.tile_pool(name="acc", bufs=1) as apool,
            tc.tile_pool(name="psB", bufs=2, space="PSUM") as psB_pool,
        ):
            iota_t = cpool.tile([P, P], F32)
            nc.sync.dma_start(iota_t[:], iota_c[:])
            idx_t = cpool.tile([P, S16], I16)
            nc.sync.dma_start(idx_t[:], idx3[:])
            colid_t = cpool.tile([P, max(NBLK3, 1)], F32)
            nc.sync.dma_start(colid_t[:], colid[:])
            csl_t = cpool.tile([P, max(NBLK3, 1)], F32)
            nc.sync.dma_start(csl_t[:], csl[:])
            zog_t = cpool.tile([P, NT3 * D], F32)
            nc.sync.dma_start(zog_t[:], zog[:])
            f0_t = cpool.tile([P, NT3], F32)
            nc.sync.dma_start(f0_t[:], f0g[:])
            g = cpool.tile([P, max(NBLK3, 1), D], BF16)
            if NBLK3 > 0:
                for off in range(0, S3, 512):
                    n = min(512, S3 - off)
                    nc.gpsimd.dma_gather(
                        g[:, off // P : (off + n) // P, :], zc[:, :],
                        idx_t[:, off // 16 : (off + n) // 16], n, n, D,
                        elem_step=D, queue_num=0,
                    )
            orow = apool.tile([P, NT3 * D], F32)

            blk0 = 0
            for i in range(NT3):
                nb = int(NB3_t[i])
                o_sl = orow[:, i * D : (i + 1) * D]
                if nb > 0:
                    sel = spool.tile([P, nb * P], BF16, tag="sel")
                    for bi in range(nb):
                        nc.vector.tensor_scalar(
                            out=sel[:, bi * P : (bi + 1) * P],
                            in0=iota_t[:],
                            scalar1=colid_t[:, blk0 + bi : blk0 + bi + 1],
                            scalar2=csl_t[:, blk0 + bi : blk0 + bi + 1],
                            op0=AL.is_equal,
                            op1=AL.mult,
                        )
                    psB = psB_pool.tile([P, P], F32, tag="B")
                    for bi in range(nb):
                        nc.tensor.matmul(
                            psB[:], lhsT=sel[:, bi * P : (bi + 1) * P],
                            rhs=g[:, blk0 + bi, :],
                            start=(bi == 0), stop=(bi == nb - 1),
                        )
                    zf = wpool.tile([P, P], F32, tag="zf")
                    nc.vector.tensor_scalar(
                        out=zf[:], in0=zog_t[:, i * D : (i + 1) * D],
                        scalar1=f0_t[:, i : i + 1], scalar2=None, op0=AL.mult,
                    )
                    nc.vector.tensor_tensor(out=o_sl, in0=zf[:], in1=psB[:],
                                            op=AL.add)
                else:
                    nc.vector.tensor_scalar(
                        out=o_sl, in0=zog_t[:, i * D : (i + 1) * D],
                        scalar1=f0_t[:, i : i + 1], scalar2=None, op0=AL.mult,
                    )
                blk0 += nb
            nc.sync.dma_start(out[:], orow[:])
    nc.compile()
    return nc


# ----------------------------------------------------------------------------
# L3 host-side layout (active tiles only)
# ----------------------------------------------------------------------------

def build_l3_layout(prep, z_np, C_np, c_np, w2):
    """Pick affected nodes, pack per-core active tiles + gather sources.

    Edges of the L3 aggregation, per affected node u (all owned by core u//R):
      real kept edge u->v: gather z[v], col u, weight -w2*c[v]
      C-row for sig c[u]:  gather C[u], col u, weight -w2*c[u]
    f0 = 1 + w2*(c*indeg + A), A = segsum over ALL edges of c[dst] by src.
    """
    N, R = prep["N"], prep["R"]
    src, dst = prep["src"], prep["dst"]
    n_cores = N_CORES
    NT = cdiv(R, P)

    tau = C_TAU
    absc = np.abs(c_np)
    kept = absc[dst] > tau
    if kept.sum() > L3_EDGE_CAP:
        thr = np.sort(absc[dst])[-L3_EDGE_CAP]
        tau = max(tau, thr)
        kept = absc[dst] > tau
    ks, kd = src[kept], dst[kept]
    signodes = np.nonzero(absc > tau)[0]

    A = np.bincount(src, weights=c_np[dst], minlength=N)
    f0_full = 1.0 + w2 * (c_np * prep["indeg"] + A)

    # per-edge entries: (owner_node u, gather_key, weight)
    # gather_key: (0, v) for z rows, (1, u) for C rows
    ent_u = np.concatenate([ks, signodes])
    ent_kind = np.concatenate([np.zeros(len(ks), np.int64),
                               np.ones(len(signodes), np.int64)])
    ent_g = np.concatenate([kd, signodes])
    ent_w = np.concatenate([-w2 * c_np[kd], -w2 * c_np[signodes]])

    owner = ent_u // R
    tl_loc = (ent_u - owner * R) // P
    col = (ent_u - owner * R) % P

    # |f0-1| below 1e-8 contributes <4e-8 abs output error — ignore, like
    # the tau edge pruning (consistent error budget).
    affected = np.unique(np.concatenate(
        [ent_u, np.nonzero(np.abs(f0_full - 1.0) > 1e-8)[0]]))

    # active local tiles per core
    act_tiles = [sorted(set(
        int((u - c * R) // P) for u in affected if u // R == c))
        for c in range(n_cores)]
    NT3 = max(1, max(len(a) for a in act_tiles))
    for a in act_tiles:
        pass  # pad handled below

    # per-core, per-active-tile edge lists
    core_data = []
    maxblk_per_slot = np.zeros(NT3, dtype=np.int64)
    for c in range(n_cores):
        tiles = act_tiles[c]
        percore = []
        for i in range(NT3):
            if i < len(tiles):
                t = tiles[i]
                m = (owner == c) & (tl_loc == t)
                percore.append((t, ent_g[m], ent_kind[m], ent_w[m], col[m]))
                maxblk_per_slot[i] = max(maxblk_per_slot[i],
                                         cdiv(len(ent_w[m]), P))
            else:
                percore.append((None, [], [], [], []))
        core_data.append(percore)

    NB3_t = maxblk_per_slot
    NBLK3 = int(NB3_t.sum())
    S3 = max(16, NBLK3 * P)

    # pack gather sources + idx per core
    z_b = bf(z_np)
    C_b = bf(C_np)
    maps = []
    M3 = 1
    packs = []
    for c in range(n_cores):
        keys = {}
        rows = []
        for (t, gg, kk, ww, cc) in core_data[c]:
            for gi, ki in zip(gg, kk):
                key = (int(ki), int(gi))
                if key not in keys:
                    keys[key] = len(rows)
                    rows.append(key)
        packs.append((keys, rows))
        M3 = max(M3, len(rows))
    M3 = max(16, M3)

    for c in range(n_cores):
        keys, rows = packs[c]
        zcarr = np.zeros((M3, D), dtype=BF)
        for r, (ki, gi) in enumerate(rows):
            zcarr[r] = C_b[gi] if ki else z_b[gi]
        idx = np.zeros(S3, dtype=np.int16)
        colid = np.full((max(NBLK3, 1), P), -1.0, dtype=np.float32)
        cslv = np.zeros((max(NBLK3, 1), P), dtype=np.float32)
        zogrid = np.zeros((P, NT3 * D), dtype=np.float32)
        f0grid = np.ones((P, NT3), dtype=np.float32)
        blk0 = 0
        tiles_used = []
        for i, (t, gg, kk, ww, cc) in enumerate(core_data[c]):
            nb = int(NB3_t[i])
            if t is not None:
                lo = c * prep["R"] + t * P
                nrow = min(P, prep["R"] - t * P)
                zogrid[:nrow, i * D : (i + 1) * D] = z_np[lo : lo + nrow]
                f0grid[:nrow, i] = f0_full[lo : lo + nrow]
                tiles_used.append((i, t, nrow))
                for e in range(len(ww)):
                    slot = blk0 * P + e
                    idx[slot] = keys[(int(kk[e]), int(gg[e]))]
                    colid[blk0 + e // P, e % P] = float(cc[e])
                    cslv[blk0 + e // P, e % P] = float(ww[e])
            blk0 += nb
        maps.append(dict(
            zc=zcarr,
            idx3=PassLayout._wrap(idx),
            colid=np.ascontiguousarray(colid.T),
            csl=np.ascontiguousarray(cslv.T),
            zog=zogrid,
            f0g=f0grid,
            _tiles=tiles_used,
        ))
    return dict(NT3=NT3, NB3_t=NB3_t, M3=M3, S3=S3, maps=maps)


# ----------------------------------------------------------------------------
# Runtime driver
# ----------------------------------------------------------------------------

def full_pipeline(inputs_np, runner, n_cores=N_CORES):
    N = inputs_np["x"].shape[0]
    prep = host_prep(inputs_np["edge_index"], N, n_cores)
    R = prep["R"]
    lay = prep["L12"]
    NT, RP = lay.NT, lay.RP
    src, dst = prep["src"], prep["dst"]
    dinv, indeg = prep["dinv"], prep["indeg"]
    gc = prep["gc"]
    if gc is None:
        gc = np.concatenate([np.arange(R, dtype=np.int64) for _ in range(n_cores)])

    Tv = float(np.asarray(inputs_np["temperature"]).reshape(-1)[0])
    wv = float(np.asarray(inputs_np["weight"]).reshape(-1)[0])
    w2 = 2.0 * wv
    b_np = np.asarray(inputs_np["b"], dtype=np.float64)

    # host: prescale x rows by dinv (src factor)
    xs = np.asarray(inputs_np["x"], dtype=np.float64) * dinv[:, None]
    xs_b = bf(xs)
    W_b = bf(inputs_np["W"])

    # L1
    nc1 = build_L1(lay, N)
    maps1 = []
    for c in range(n_cores):
        lc = lay.cores[c]
        lo = c * R
        xsT = np.zeros((P, RP), dtype=BF)
        xsT[:, gc[lo : lo + R]] = xs_b[lo : lo + R].T
        maps1.append(dict(
            x=xs_b, W=W_b,
            idxA=lc["idxA"], idxB=lc["idxB"],
            colid=bf(lc["colid"]), xsT=xsT,
        ))
    res1 = runner(nc1, maps1, ["zT"])

    # host: z = dinv * z~ + b ; q ; segq
    zt = np.concatenate(
        [np.asarray(res1[c]["zT"], dtype=np.float64)[:, gc[c * R:(c + 1) * R]].T
         for c in range(n_cores)], axis=0)  # [N, D]
    z_np = zt * dinv[:, None] + b_np[None, :]
    z_b = bf(z_np)
    q = np.einsum("nd,nd->n", z_np, z_np)
    segq = np.bincount(dst, weights=q[src], minlength=N)

    # L2
    nc2 = build_L2(lay, N)
    maps2 = []
    for c in range(n_cores):
        lc = lay.cores[c]
        maps2.append(dict(
            z=z_b,
            idxA=lc["idxA"], idxB=lc["idxB"],
            colid=bf(lc["colid"]),
        ))
    res2 = runner(nc2, maps2, ["CT"])

    C_np = np.concatenate(
        [np.asarray(res2[c]["CT"], dtype=np.float64)[:, gc[c * R:(c + 1) * R]].T
         for c in range(n_cores)], axis=0)  # [N, D]

    # host: E, softmax, c
    E = segq + indeg * q - 2.0 * np.einsum("nd,nd->n", z_np, C_np)
    a = -E / Tv
    m = a.max()
    ex = np.exp(a - m)
    S = ex.sum()
    p = ex / S
    logp = a - m - np.log(S)
    H = -np.sum(p * logp)
    c_np = (1.0 / Tv) * p * (logp + H)

    l3 = build_l3_layout(prep, z_np, C_np, c_np, w2)
    nc3 = build_L3(l3["NT3"], l3["NB3_t"], l3["M3"], l3["S3"])
    maps3 = [{k: v for k, v in m.items() if not k.startswith("_")}
             for m in l3["maps"]]
    res3 = runner(nc3, maps3, ["out"])

    out = np.ascontiguousarray(z_np.astype(np.float32))
    for c in range(n_cores):
        o = np.asarray(res3[c]["out"], dtype=np.float32)
        for (i, t, nrow) in l3["maps"][c]["_tiles"]:
            lo = c * R + t * P
            out[lo : lo + nrow] = o[:nrow, i * D : (i + 1) * D]
    return out


# ----------------------------------------------------------------------------
# Entry point + timing
# ----------------------------------------------------------------------------

TRACE = False
TIME_REPS = 0
LAST_EXEC_TIMES = []
STUB_TIMES = []
TRACE_TIMES = []


def _hw_runner(nc, in_maps, out_names):
    from concourse.bass_utils import run_bass_kernel_spmd
    res = run_bass_kernel_spmd(nc, in_maps, core_ids=list(range(len(in_maps))),
                               trace=TRACE)
    if TRACE and res.exec_time_ns:
        TRACE_TIMES.append(res.exec_time_ns)
    if TIME_REPS:
        k_ns, s_ns = _time_launch_pair(nc, _build_stub(nc), in_maps,
                                       max(3, TIME_REPS))
        LAST_EXEC_TIMES.append(k_ns)
        STUB_TIMES.append(s_ns)
    return res.results


KINNER = int(_os.environ.get("KINNER", "8"))


def _time_launch_pair(nc, nc_stub, in_maps, reps):
    """Interleave batches of kernel and stub executions so the ~80ms axon
    dispatch overhead (and its drift) cancels out of the difference; the
    inner batch of KINNER launches divides per-sample noise by KINNER."""
    import jax, time as _time
    fn_k, in_k, zo_k = _make_sharded(nc, in_maps)
    fn_s, in_s, zo_s = _make_sharded(nc_stub, in_maps)
    wk, ws = [], []
    for r in range(reps + 1):
        t0 = _time.perf_counter()
        for _ in range(KINNER):
            outs = fn_k(*in_k, *zo_k)
        jax.block_until_ready(outs)
        t1 = _time.perf_counter()
        for _ in range(KINNER):
            outs = fn_s(*in_s, *zo_s)
        jax.block_until_ready(outs)
        t2 = _time.perf_counter()
        if r:
            wk.append((t1 - t0) * 1e9 / KINNER)
            ws.append((t2 - t1) * 1e9 / KINNER)
    return min(wk), min(ws)


def run_sim(nc, in_maps, out_names):
    from concourse.bass_interp import MultiCoreSim
    n = len(in_maps)
    sim = MultiCoreSim(nc, num_cores=n)
    for c in range(n):
        for kk, v in in_maps[c].items():
            sim.cores[c].tensor(kk)[:] = v
    sim.simulate(check_with_hw=False)
    return [{kk: np.array(sim.cores[c].tensor(kk)) for kk in out_names}
            for c in range(n)]


def kernel(**inputs):
    inputs_np = {k: np.asarray(v) for k, v in inputs.items()}
    out = full_pipeline(inputs_np, _hw_runner)
    return out.astype(np.float32)


def _make_sharded(nc, in_maps):
    import jax
    import concourse.mybir as _mybir
    from concourse import bass2jax as b2j
    from jax.experimental.shard_map import shard_map
    from jax.sharding import Mesh, PartitionSpec, NamedSharding

    b2j.install_neuronx_cc_hook()
    n_cores = len(in_maps)
    partition_name = nc.partition_id_tensor.name if nc.partition_id_tensor else None
    in_names, out_names, out_avals, zero_outs = [], [], [], []
    for alloc in nc.m.functions[0].allocations:
        if not isinstance(alloc, _mybir.MemoryLocationSet):
            continue
        name = alloc.memorylocations[0].name
        if alloc.kind == "ExternalInput":
            if name != partition_name:
                in_names.append(name)
        elif alloc.kind == "ExternalOutput":
            shape = tuple(alloc.tensor_shape)
            dtype = _mybir.dt.np(alloc.dtype)
            out_names.append(name)
            out_avals.append(jax.core.ShapedArray(shape, dtype))
            zero_outs.append(np.zeros(shape, dtype))
    n_params = len(in_names)
    all_in = in_names + out_names
    if partition_name is not None:
        all_in = all_in + [partition_name]

    def _body(*args):
        operands = list(args)
        if partition_name is not None:
            operands.append(b2j.partition_id_tensor())
        outs = b2j._bass_exec_p.bind(
            *operands,
            out_avals=tuple(out_avals),
            in_names=tuple(all_in),
            out_names=tuple(out_names),
            lowering_input_output_aliases=(),
            sim_require_finite=True,
            sim_require_nnan=True,
            nc=nc,
        )
        return tuple(outs)

    devices = jax.devices()[:n_cores]
    mesh = Mesh(np.asarray(devices), ("core",))
    spec = PartitionSpec("core")
    in_specs = (spec,) * (n_params + len(out_names))
    out_specs = (spec,) * len(out_names)
    fn = jax.jit(
        shard_map(_body, mesh=mesh, in_specs=in_specs, out_specs=out_specs,
                  check_rep=False),
        keep_unused=True,
    )
    sh = NamedSharding(mesh, spec)
    concat_in = [
        jax.device_put(
            np.concatenate([np.asarray(in_maps[c][nm]) for c in range(n_cores)],
                           axis=0),
            sh,
        )
        for nm in in_names
    ]
    concat_zero = [
        jax.device_put(np.zeros((n_cores * z.shape[0], *z.shape[1:]), z.dtype), sh)
        for z in zero_outs
    ]
    return fn, concat_in, concat_zero


def _time_launch(nc, in_maps, reps):
    import jax, time as _time
    fn, concat_in, concat_zero = _make_sharded(nc, in_maps)
    walls = []
    for _ in range(reps + 1):
        t0 = _time.perf_counter()
        outs = fn(*concat_in, *concat_zero)
        jax.block_until_ready(outs)
        walls.append((_time.perf_counter() - t0) * 1e9)
    return min(walls[1:]) if len(walls) > 1 else walls[0]


def _build_stub(nc_ref):
    import concourse.mybir as _mybir
    nc = bacc.Bacc("TRN2", target_bir_lowering=False, debug=False)
    outs = []
    for alloc in nc_ref.m.functions[0].allocations:
        if not isinstance(alloc, _mybir.MemoryLocationSet):
            continue
        ml = alloc.memorylocations[0]
        if alloc.kind == "ExternalInput":
            if nc_ref.partition_id_tensor and ml.name == nc_ref.partition_id_tensor.name:
                continue
            nc.dram_tensor(ml.name, list(alloc.tensor_shape), alloc.dtype,
                           kind="ExternalInput")
        elif alloc.kind == "ExternalOutput":
            outs.append(nc.dram_tensor(ml.name, list(alloc.tensor_shape),
                                       alloc.dtype, kind="ExternalOutput"))
    with tile.TileContext(nc) as tc:
        with tc.tile_pool(name="w", bufs=1) as wp:
            for o in outs:
                t = wp.tile([1, 16], o.dtype, tag="t")
                nc.vector.memset(t[:], 0)
                sl = tuple([slice(0, 1)] * (len(o.shape) - 1)
                           + [slice(0, min(16, o.shape[-1]))])
                nc.sync.dma_start(o[sl], t[0:1, 0:min(16, o.shape[-1])])
    nc.compile()
    return nc


# revision 35
# speedup vs baseline: 1.5823x; 1.5823x over previous
"""EntropicLayer (GCN conv + entropy gradient) on 8 trn2 NeuronCores — v3.

3-launch SPMD, node-partitioned (dst-sharded edges), device does all
O(E*D) message passing and O(N*D^2) transforms; host does per-node
scalar glue between launches (dinv/bias/q/E/softmax/c) and layout.

  L1: gather xs=x*dinv_src (bf16) per dst-owned edge; per dst tile
      aggT[feat,node] = sum_blk g_blk^T @ sel_blk (unweighted one-hot sel,
      one broadcast is_equal per batch) + xsT_own tile add (self-loops);
      z~T = W^T @ aggT; single wide store.
      Host: z = dinv*z~ + b; q = ||z||^2; segq; E prep.
  L2: gather z (bf16) per edge; CT[feat,node] aggregation; single store.
      Host: C; zC; E = segq + indeg*q - 2 zC; softmax -> c (fp64);
      prune |c| <= tau; f0 = 1 + 2w(c*indeg + A); pack active tiles.
  L3: only tiles containing affected nodes (~dozens): gather packed
      z/C rows; weighted sel (-2w*c); psB aggregation node-major;
      o = z*f0 + psB; host scatters active tiles into out = z.
All matmuls bf16 (4x fp32 PE rate), fp32 PSUM accumulation.
"""

import math
import numpy as np
import ml_dtypes

import concourse.bass as bass
import concourse.bacc as bacc
import concourse.mybir as mybir
import concourse.tile as tile

P = 128
D = 128
F32 = mybir.dt.float32
BF16 = mybir.dt.bfloat16
I16 = mybir.dt.int16
AL = mybir.AluOpType
ACTF = mybir.ActivationFunctionType
AX = mybir.AxisListType
BF = ml_dtypes.bfloat16

N_NODES = 50000
N_CORES = 8

import os as _os
GCH = int(_os.environ.get("KGCH", "512"))
DDS = int(_os.environ.get("KDDS", "65536"))
TB = int(_os.environ.get("KTB", "4"))
KCW = int(_os.environ.get("KCW", "32"))
KNOID = _os.environ.get("KNOID", "0") == "1"
C_TAU = 1e-7  # |c| threshold for L3 edge pruning
L3_EDGE_CAP = 32768  # adaptive tau fallback cap


def cdiv(a, b):
    return (a + b - 1) // b


def bf(a):
    return np.ascontiguousarray(np.asarray(a).astype(BF))


# ----------------------------------------------------------------------------
# Host-side layout prep
# ----------------------------------------------------------------------------

class PassLayout:
    """Per-pass slot/block layout, uniform across cores.

    Aggregation windows are CW node-columns wide (CW <= 128): each block is
    [128 slots x CW cols], shrinking one-hot sel area by 128/CW while PSUM
    windows of 4 CW-tiles share one [128,128] accumulation tile.
    """

    def __init__(self, N, n_cores, TB=TB, cw=KCW):
        self.N = N
        self.n_cores = n_cores
        self.R = N // n_cores
        self.NT = cdiv(self.R, P)
        self.RP = self.NT * P
        self.CW = cw
        self.WPT = P // cw  # windows per 128-tile
        self.NW = self.NT * self.WPT
        self.HALF = min(32768, (N + 1) // 2)
        assert self.HALF <= 32768 and N - self.HALF <= 32768
        self.TB = TB
        self.batches = []
        t = 0
        while t < self.NT:
            ntk = min(TB, self.NT - t)
            self.batches.append((t, ntk))
            t += ntk

    def build(self, gidx, cnode, min_blocks=1, gc=None):
        """gidx: gather-node per edge; cnode: scatter-node per edge.

        gc: optional [N] grid-column map (node -> column in its owner's
        [P, RP] output grid), from host bin-packing. Default: node order.
        """
        N, n_cores, R, NW, HALF = self.N, self.n_cores, self.R, self.NW, self.HALF
        CW, WPT = self.CW, self.WPT
        gidx = np.asarray(gidx, dtype=np.int64)
        cnode = np.asarray(cnode, dtype=np.int64)
        owner = cnode // R
        if gc is None:
            loc = cnode - owner * R
            wl = loc // CW  # window index
            col = loc % CW
        else:
            gcp = gc[cnode]
            wl = (gcp // P) * WPT + (gcp % P) // CW
            col = gcp % CW
        h = (gidx >= HALF).astype(np.int64)

        key = (owner * NW + wl) * 2 + h
        counts = np.bincount(key, minlength=n_cores * NW * 2).reshape(n_cores, NW, 2)
        BA_w = np.maximum(min_blocks, (counts[:, :, 0].max(axis=0) + P - 1) // P).astype(np.int64)
        BB_w = np.maximum(min_blocks, (counts[:, :, 1].max(axis=0) + P - 1) // P).astype(np.int64)
        self.BA_w, self.BB_w = BA_w, BB_w
        astart = np.zeros(NW + 1, dtype=np.int64)
        astart[1:] = np.cumsum(BA_w * P)
        bstart = np.zeros(NW + 1, dtype=np.int64)
        bstart[1:] = np.cumsum(BB_w * P)
        self.astart, self.bstart = astart, bstart
        self.SA = int(astart[-1])
        self.SB = int(bstart[-1])
        ABLK0 = np.zeros(NW, dtype=np.int64)
        BBLK0 = np.zeros(NW, dtype=np.int64)
        base_blk = []
        acc = 0
        maxb = 0
        for (t0, ntk) in self.batches:
            base_blk.append(acc)
            a = acc
            for j in range(ntk * WPT):
                ABLK0[t0 * WPT + j] = a
                a += int(BA_w[t0 * WPT + j])
            for j in range(ntk * WPT):
                BBLK0[t0 * WPT + j] = a
                a += int(BB_w[t0 * WPT + j])
            maxb = max(maxb, a - acc)
            acc = a
        self.ABLK0, self.BBLK0 = ABLK0, BBLK0
        self.base_blk = base_blk + [acc]
        self.NBLK = max(acc, 1)
        self.MAXB = max(maxb, 1)
        self.SA = max(self.SA, 16)
        self.SB = max(self.SB, 16)

        order = np.argsort(key, kind="stable")
        ks = key[order]
        group_start = np.zeros(n_cores * NW * 2, dtype=np.int64)
        cnt_flat = np.bincount(key, minlength=n_cores * NW * 2)
        group_start[1:] = np.cumsum(cnt_flat)[:-1]
        pos = np.arange(len(order)) - group_start[ks]

        og, oc, ow, oh, ocol = (gidx[order], owner[order], wl[order], h[order], col[order])
        oeid = order  # original edge ordinal per sorted slot

        cores = []
        for c in range(n_cores):
            m = oc == c
            cg, cww, ch, ccol, cpos, ceid = og[m], ow[m], oh[m], ocol[m], pos[m], oeid[m]

            idxA = np.zeros(self.SA, dtype=np.int16)
            idxB = np.zeros(self.SB, dtype=np.int16)
            colid = np.full((self.NBLK, P), -1.0, dtype=np.float32)
            eord = np.full((self.NBLK, P), -1, dtype=np.int64)

            for half, (idxarr, wstart, wblk0, off) in enumerate(
                [(idxA, astart, ABLK0, 0), (idxB, bstart, BBLK0, HALF)]
            ):
                mm = ch == half
                ww, ppos, gg, ccc, ee = cww[mm], cpos[mm], cg[mm], ccol[mm], ceid[mm]
                stream = wstart[ww] + ppos
                idxarr[stream] = (gg - off).astype(np.int16)
                i = ppos // P
                p = ppos % P
                blk = wblk0[ww] + i
                colid[blk, p] = ccc.astype(np.float32)
                eord[blk, p] = ee

            cores.append(
                dict(
                    idxA=self._wrap(idxA),
                    idxB=self._wrap(idxB),
                    colid=np.ascontiguousarray(colid.T),  # [P, NBLK] f32
                    eord=eord,  # [NBLK, P] edge ordinal or -1
                )
            )
        self.cores = cores
        return self

    @staticmethod
    def _wrap(flat):
        S = len(flat)
        assert S % 16 == 0
        arr = flat.reshape(S // 16, 16).T  # [16, S/16]
        return np.ascontiguousarray(np.tile(arr, (8, 1)))  # [128, S/16]

    def permute_edges(self, core, edge_vec, padval, dtype=np.float32):
        """[P, NBLK] array with edge_vec[eord] (padval on pads)."""
        eord = self.cores[core]["eord"]
        out = np.full(eord.shape, padval, dtype=np.float64)
        m = eord >= 0
        out[m] = edge_vec[eord[m]]
        return np.ascontiguousarray(out.T.astype(dtype))  # [P, NBLK]


def _pack_core(cntA, cntB, capA, capB, CW):
    """Greedy-pack one core's nodes into windows under (A, B) capacities.
    Returns gc [R] grid column per local node, or None if infeasible."""
    NW = len(capA)
    R = len(cntA)
    order = np.argsort(-(cntA + cntB), kind="stable")
    sumA = np.zeros(NW, dtype=np.int64)
    sumB = np.zeros(NW, dtype=np.int64)
    nn = np.zeros(NW, dtype=np.int64)
    assign_w = np.empty(R, dtype=np.int64)
    assign_j = np.empty(R, dtype=np.int64)
    for v in order:
        a, b = int(cntA[v]), int(cntB[v])
        rA = capA - sumA - a
        rB = capB - sumB - b
        feas = (rA >= 0) & (rB >= 0) & (nn < CW)
        if not feas.any():
            return None
        score = np.where(feas, rA + rB, -1)
        w = int(np.argmax(score))
        assign_w[v] = w
        assign_j[v] = nn[w]
        sumA[w] += a
        sumB[w] += b
        nn[w] += 1
    WPT = P // CW
    return (assign_w // WPT) * P + (assign_w % WPT) * CW + assign_j


def _pack_all(src, dst, N, n_cores, lay):
    """Shared bimodal window-capacity profile + per-core packing."""
    R = N // n_cores
    NW, CW = lay.NW, lay.CW
    isA = (src < lay.HALF).astype(np.int64)
    cA = np.bincount(dst, weights=isA, minlength=N).astype(np.int64)
    cB = np.bincount(dst, weights=1 - isA, minlength=N).astype(np.int64)
    maxA = max(int(cA[c * R:(c + 1) * R].sum()) for c in range(n_cores))
    maxB = max(int(cB[c * R:(c + 1) * R].sum()) for c in range(n_cores))
    for margin in (4, 8, 16, 32):
        n2A = min(NW, max(0, cdiv(maxA + margin * 128 - 124 * NW, 128)))
        n2B = min(NW, max(0, cdiv(maxB + margin * 128 - 124 * NW, 128)))
        capA = np.full(NW, 124, dtype=np.int64)
        capA[:n2A] = 252
        capB = np.full(NW, 124, dtype=np.int64)
        capB[NW - n2B:] = 252  # stagger A/B heavy windows
        if capA.sum() < maxA or capB.sum() < maxB:
            continue
        gc = np.empty(N, dtype=np.int64)
        ok = True
        for c in range(n_cores):
            lo = c * R
            g = _pack_core(cA[lo:lo + R], cB[lo:lo + R], capA, capB, CW)
            if g is None:
                ok = False
                break
            gc[lo:lo + R] = g
        if ok:
            return gc
    return None


def host_prep(edge_index, N, n_cores):
    src = np.asarray(edge_index[0], dtype=np.int64)
    dst = np.asarray(edge_index[1], dtype=np.int64)
    E = len(src)

    deg = np.bincount(dst, minlength=N).astype(np.float64) + 1.0
    dinv = deg ** -0.5
    indeg = np.bincount(dst, minlength=N).astype(np.float64)

    R = N // n_cores
    lay = PassLayout(N, n_cores)
    gc = _pack_all(src, dst, N, n_cores, lay) if _os.environ.get(
        "KPACK", "0") == "1" else None
    L12 = lay.build(src, dst, gc=gc)
    return dict(src=src, dst=dst, dinv=dinv, indeg=indeg,
                L12=L12, gc=gc, R=R, N=N, E=E)


# ----------------------------------------------------------------------------
# Device builders
# ----------------------------------------------------------------------------

IOTA_BF = np.tile(np.arange(P, dtype=np.float32), (P, 1)).astype(BF)


def _gather_batches(nc, lay, gpool, ipool, x_dram, idxA, idxB):
    """Yield (k, t0, ntk, gtile) with both A and B regions gathered (bf16).

    idx slices are DMA-loaded per batch (pipelined) so the first gather
    starts ~5us earlier than with monolithic idx loads."""
    HALF = lay.HALF
    N = lay.N
    aofs = 0
    bofs = 0
    qn = 0
    for k, (t0, ntk) in enumerate(lay.batches):
        g = gpool.tile([P, lay.MAXB, D], BF16, tag="gat")
        w0, w1 = t0 * lay.WPT, (t0 + ntk) * lay.WPT
        nAblk = int(sum(lay.BA_w[w0:w1]))
        nBblk = int(sum(lay.BB_w[w0:w1]))
        nA = nAblk * P
        nB = nBblk * P
        idx_t = ipool.tile([P, (nA + nB) // 16], I16, tag="idx")
        nc.sync.dma_start(idx_t[:, 0 : nA // 16],
                          idxA[:, aofs : aofs + nA // 16])
        nc.sync.dma_start(idx_t[:, nA // 16 : (nA + nB) // 16],
                          idxB[:, bofs : bofs + nB // 16])
        for off in range(0, nA, GCH):
            n = min(GCH, nA - off)
            nc.gpsimd.dma_gather(
                g[:, off // P : (off + n) // P, :],
                x_dram[0:HALF, :],
                idx_t[:, off // 16 : (off + n) // 16],
                n, n, D, elem_step=D, queue_num=qn,
            )
            qn = 1 - qn
        for off in range(0, nB, GCH):
            n = min(GCH, nB - off)
            nc.gpsimd.dma_gather(
                g[:, nAblk + off // P : nAblk + (off + n) // P, :],
                x_dram[HALF:N, :],
                idx_t[:, (nA + off) // 16 : (nA + off + n) // 16],
                n, n, D, elem_step=D, queue_num=qn,
            )
            qn = 1 - qn
        aofs += nA // 16
        bofs += nB // 16
        yield k, t0, ntk, g


def _sel_batch(nc, spool, colid_t, iota_t, blk0, NB, CW):
    """Unweighted one-hot sel (bf16) for NB consecutive blocks, one DVE op."""
    sel = spool.tile([P, NB * CW], BF16, tag="sel")
    s3 = sel[:].rearrange("p (b f) -> p b f", f=CW)
    iap = iota_t[:, 0:CW]
    iota3 = bass.AP(iap.tensor, iap.offset, [iap.ap[0], [0, NB], iap.ap[1]])
    nc.vector.tensor_tensor(
        out=s3,
        in0=colid_t[:, blk0 : blk0 + NB].to_broadcast([P, NB, CW]),
        in1=iota3,
        op=AL.is_equal,
    )
    return sel


def _wblocks(lay, t, k):
    """Per 128-tile t in batch k: per-window lists of (buf, blk)."""
    base = lay.base_blk[k]
    out = []
    for wi in range(lay.WPT):
        w = t * lay.WPT + wi
        wb = []
        for i in range(int(lay.BA_w[w])):
            wb.append((int(lay.ABLK0[w]) - base + i, int(lay.ABLK0[w]) + i))
        for i in range(int(lay.BB_w[w])):
            wb.append((int(lay.BBLK0[w]) - base + i, int(lay.BBLK0[w]) + i))
        out.append(wb)
    return out


def build_L1(lay, N):
    nc = bacc.Bacc("TRN2", target_bir_lowering=False, debug=False,
                   dynamic_dma_scratch_size=DDS, num_swdge_queues=2)
    NT, RP, NBLK = lay.NT, lay.RP, lay.NBLK
    SA16, SB16 = lay.SA // 16, lay.SB // 16

    x = nc.dram_tensor("x", [N, D], BF16, kind="ExternalInput")
    Wm = nc.dram_tensor("W", [D, D], BF16, kind="ExternalInput")
    idxA = nc.dram_tensor("idxA", [P, SA16], I16, kind="ExternalInput")
    idxB = nc.dram_tensor("idxB", [P, SB16], I16, kind="ExternalInput")
    colid = nc.dram_tensor("colid", [P, NBLK], BF16, kind="ExternalInput")
    xsT = nc.dram_tensor("xsT", [P, RP], BF16, kind="ExternalInput")
    iota_c = nc.inline_tensor(IOTA_BF, "iota_c")
    ident_c = nc.inline_tensor(np.eye(P, dtype=np.float32).astype(BF), "ident_c")

    zT_out = nc.dram_tensor("zT", [P, RP], F32, kind="ExternalOutput")
    CW = lay.CW

    with tile.TileContext(nc) as tc:
        with (
            tc.tile_pool(name="const", bufs=1) as cpool,
            tc.tile_pool(name="gat", bufs=2) as gpool,
            tc.tile_pool(name="idx", bufs=3) as ipool,
            tc.tile_pool(name="sel", bufs=2) as spool,
            tc.tile_pool(name="work", bufs=3) as wpool,
            tc.tile_pool(name="acc", bufs=1) as apool,
            tc.tile_pool(name="psA", bufs=2, space="PSUM") as psA_pool,
            tc.tile_pool(name="psZ", bufs=2, space="PSUM") as psZ_pool,
        ):
            # const loads on DVE/Act HWDGE queues so SP starts with the
            # batch-0 idx slices (first gather ~1us instead of ~9us)
            colid_t = cpool.tile([P, NBLK], BF16)
            nc.scalar.dma_start(colid_t[:], colid[:])
            iota_t = cpool.tile([P, P], BF16)
            nc.scalar.dma_start(iota_t[:], iota_c[:])
            ident_t = cpool.tile([P, P], BF16)
            nc.scalar.dma_start(ident_t[:], ident_c[:])
            W_t = cpool.tile([P, P], BF16)
            nc.scalar.dma_start(W_t[:], Wm[:])
            xsT_t = cpool.tile([P, RP], BF16)
            nc.scalar.dma_start(xsT_t[:], xsT[:])
            zrow = apool.tile([P, RP], F32)

            for k, t0, ntk, g in _gather_batches(nc, lay, gpool, ipool, x,
                                                 idxA, idxB):
                base = lay.base_blk[k]
                NBk = lay.base_blk[k + 1] - base
                sel = _sel_batch(nc, spool, colid_t, iota_t, base, NBk, CW)
                for j in range(ntk):
                    t = t0 + j
                    psA = psA_pool.tile([P, P], F32, tag="agg")
                    for wi, blocks in enumerate(_wblocks(lay, t, k)):
                        win = psA[:, wi * CW : (wi + 1) * CW]
                        c0 = t * P + wi * CW
                        nb = len(blocks)
                        if not KNOID:
                            # self-loop row add via identity matmul (starts group)
                            nc.tensor.matmul(
                                win, lhsT=ident_t[:], rhs=xsT_t[:, c0 : c0 + CW],
                                start=True, stop=False,
                            )
                        for bi, (buf, blk) in enumerate(blocks):
                            nc.tensor.matmul(
                                win, lhsT=g[:, buf, :],
                                rhs=sel[:, buf * CW : (buf + 1) * CW],
                                start=(KNOID and bi == 0), stop=(bi == nb - 1),
                            )
                    aggT_s = wpool.tile([P, P], BF16, tag="aggTs")
                    if KNOID:
                        nc.vector.tensor_tensor(
                            out=aggT_s[:], in0=psA[:],
                            in1=xsT_t[:, t * P : (t + 1) * P], op=AL.add,
                        )
                    else:
                        nc.any.tensor_copy(out=aggT_s[:], in_=psA[:])
                    psZ = psZ_pool.tile([P, P], F32, tag="zT")
                    nc.tensor.matmul(psZ[:], lhsT=W_t[:], rhs=aggT_s[:],
                                     start=True, stop=True)
                    nc.any.tensor_copy(out=zrow[:, t * P : (t + 1) * P], in_=psZ[:])
                nc.sync.dma_start(
                    zT_out[:, t0 * P : (t0 + ntk) * P],
                    zrow[:, t0 * P : (t0 + ntk) * P])
    nc.compile()
    return nc


def build_L2(lay, N):
    nc = bacc.Bacc("TRN2", target_bir_lowering=False, debug=False,
                   dynamic_dma_scratch_size=DDS, num_swdge_queues=2)
    NT, RP, NBLK = lay.NT, lay.RP, lay.NBLK
    SA16, SB16 = lay.SA // 16, lay.SB // 16

    z = nc.dram_tensor("z", [N, D], BF16, kind="ExternalInput")
    idxA = nc.dram_tensor("idxA", [P, SA16], I16, kind="ExternalInput")
    idxB = nc.dram_tensor("idxB", [P, SB16], I16, kind="ExternalInput")
    colid = nc.dram_tensor("colid", [P, NBLK], BF16, kind="ExternalInput")
    iota_c = nc.inline_tensor(IOTA_BF, "iota_c")

    CT_out = nc.dram_tensor("CT", [P, RP], F32, kind="ExternalOutput")

    with tile.TileContext(nc) as tc:
        with (
            tc.tile_pool(name="const", bufs=1) as cpool,
            tc.tile_pool(name="gat", bufs=2) as gpool,
            tc.tile_pool(name="idx", bufs=3) as ipool,
            tc.tile_pool(name="sel", bufs=2) as spool,
            tc.tile_pool(name="acc", bufs=1) as apool,
            tc.tile_pool(name="psC", bufs=2, space="PSUM") as psC_pool,
        ):
            colid_t = cpool.tile([P, NBLK], BF16)
            nc.scalar.dma_start(colid_t[:], colid[:])
            iota_t = cpool.tile([P, P], BF16)
            nc.scalar.dma_start(iota_t[:], iota_c[:])
            Crow = apool.tile([P, RP], F32)
            CW = lay.CW

            for k, t0, ntk, g in _gather_batches(nc, lay, gpool, ipool, z,
                                                 idxA, idxB):
                base = lay.base_blk[k]
                NBk = lay.base_blk[k + 1] - base
                sel = _sel_batch(nc, spool, colid_t, iota_t, base, NBk, CW)
                for j in range(ntk):
                    t = t0 + j
                    psC = psC_pool.tile([P, P], F32, tag="C")
                    for wi, blocks in enumerate(_wblocks(lay, t, k)):
                        win = psC[:, wi * CW : (wi + 1) * CW]
                        nb = len(blocks)
                        for bi, (buf, blk) in enumerate(blocks):
                            nc.tensor.matmul(
                                win, lhsT=g[:, buf, :],
                                rhs=sel[:, buf * CW : (buf + 1) * CW],
                                start=(bi == 0), stop=(bi == nb - 1),
                            )
                    nc.any.tensor_copy(out=Crow[:, t * P : (t + 1) * P], in_=psC[:])
                nc.sync.dma_start(
                    CT_out[:, t0 * P : (t0 + ntk) * P],
                    Crow[:, t0 * P : (t0 + ntk) * P])
    nc.compile()
    return nc


def build_L3(NT3, NB3_t, M3, S3):
    """Active-tile combine: o = z*f0 + sum_blk sel_w^T @ g  (node-major).

    NT3: active tiles per core (uniform); NB3_t[i]: blocks for slot i;
    M3: packed gather source rows; S3: total gather slots (mult of 16).
    """
    nc = bacc.Bacc("TRN2", target_bir_lowering=False, debug=False,
                   dynamic_dma_scratch_size=16384, num_swdge_queues=1)
    NBLK3 = int(sum(NB3_t))
    S16 = S3 // 16

    zc = nc.dram_tensor("zc", [M3, D], BF16, kind="ExternalInput")
    idx3 = nc.dram_tensor("idx3", [P, S16], I16, kind="ExternalInput")
    colid = nc.dram_tensor("colid", [P, max(NBLK3, 1)], F32, kind="ExternalInput")
    csl = nc.dram_tensor("csl", [P, max(NBLK3, 1)], F32, kind="ExternalInput")
    zog = nc.dram_tensor("zog", [P, NT3 * D], F32, kind="ExternalInput")
    f0g = nc.dram_tensor("f0g", [P, NT3], F32, kind="ExternalInput")
    iota_c = nc.inline_tensor(
        np.tile(np.arange(P, dtype=np.float32), (P, 1)), "iota_c")

    out = nc.dram_tensor("out", [P, NT3 * D], F32, kind="ExternalOutput")

    with tile.TileContext(nc) as tc:
        with (
            tc.tile_pool(name="const", bufs=1) as cpool,
            tc.tile_pool(name="sel", bufs=2) as spool,
            tc.tile_pool(name="work", bufs=3) as wpool,
            tc.tile_pool(name="acc", bufs=1) as apool,
            tc.tile_pool(name="psB", bufs=2, space="PSUM") as psB_pool,
        ):
            idx_t = cpool.tile([P, S16], I16)
            nc.sync.dma_start(idx_t[:], idx3[:])
            colid_t = cpool.tile([P, max(NBLK3, 1)], F32)
            nc.sync.dma_start(colid_t[:], colid[:])
            csl_t = cpool.tile([P, max(NBLK3, 1)], F32)
            nc.sync.dma_start(csl_t[:], csl[:])
            iota_t = cpool.tile([P, P], F32)
            nc.sync.dma_start(iota_t[:], iota_c[:])
            zog_t = cpool.tile([P, NT3 * D], F32)
            nc.scalar.dma_start(zog_t[:], zog[:])
            f0_t = cpool.tile([P, NT3], F32)
            nc.scalar.dma_start(f0_t[:], f0g[:])
            g = cpool.tile([P, max(NBLK3, 1), D], BF16)
            if NBLK3 > 0:
                for off in range(0, S3, 512):
                    n = min(512, S3 - off)
                    nc.gpsimd.dma_gather(
                        g[:, off // P : (off + n) // P, :], zc[:, :],
                        idx_t[:, off // 16 : (off + n) // 16], n, n, D,
                        elem_step=D, queue_num=0,
                    )
            orow = apool.tile([P, NT3 * D], F32)

            blk0 = 0
            for i in range(NT3):
                nb = int(NB3_t[i])
                o_sl = orow[:, i * D : (i + 1) * D]
                if nb > 0:
                    sel = spool.tile([P, nb * P], BF16, tag="sel")
                    for bi in range(nb):
                        nc.vector.tensor_scalar(
                            out=sel[:, bi * P : (bi + 1) * P],
                            in0=iota_t[:],
                            scalar1=colid_t[:, blk0 + bi : blk0 + bi + 1],
                            scalar2=csl_t[:, blk0 + bi : blk0 + bi + 1],
                            op0=AL.is_equal,
                            op1=AL.mult,
                        )
                    psB = psB_pool.tile([P, P], F32, tag="B")
                    for bi in range(nb):
                        nc.tensor.matmul(
                            psB[:], lhsT=sel[:, bi * P : (bi + 1) * P],
                            rhs=g[:, blk0 + bi, :],
                            start=(bi == 0), stop=(bi == nb - 1),
                        )
                    zf = wpool.tile([P, P], F32, tag="zf")
                    nc.vector.tensor_scalar(
                        out=zf[:], in0=zog_t[:, i * D : (i + 1) * D],
                        scalar1=f0_t[:, i : i + 1], scalar2=None, op0=AL.mult,
                    )
                    nc.vector.tensor_tensor(out=o_sl, in0=zf[:], in1=psB[:],
                                            op=AL.add)
                else:
                    nc.vector.tensor_scalar(
                        out=o_sl, in0=zog_t[:, i * D : (i + 1) * D],
                        scalar1=f0_t[:, i : i + 1], scalar2=None, op0=AL.mult,
                    )
                nc.sync.dma_start(out[:, i * D : (i + 1) * D], o_sl)
                blk0 += nb
    nc.compile()
    return nc


# ----------------------------------------------------------------------------
# L3 host-side layout (active tiles only)
# ----------------------------------------------------------------------------

def build_l3_layout(prep, z_np, C_np, c_np, w2):
    """Pick affected nodes, pack per-core active tiles + gather sources.

    Edges of the L3 aggregation, per affected node u (all owned by core u//R):
      real kept edge u->v: gather z[v], col u, weight -w2*c[v]
      C-row for sig c[u]:  gather C[u], col u, weight -w2*c[u]
    f0 = 1 + w2*(c*indeg + A), A = segsum over ALL edges of c[dst] by src.
    """
    N, R = prep["N"], prep["R"]
    src, dst = prep["src"], prep["dst"]
    n_cores = N_CORES
    NT = cdiv(R, P)

    tau = C_TAU
    absc = np.abs(c_np)
    kept = absc[dst] > tau
    if kept.sum() > L3_EDGE_CAP:
        thr = np.sort(absc[dst])[-L3_EDGE_CAP]
        tau = max(tau, thr)
        kept = absc[dst] > tau
    ks, kd = src[kept], dst[kept]
    signodes = np.nonzero(absc > tau)[0]

    A = np.bincount(src, weights=c_np[dst], minlength=N)
    f0_full = 1.0 + w2 * (c_np * prep["indeg"] + A)

    # per-edge entries: (owner_node u, gather_key, weight)
    # gather_key: (0, v) for z rows, (1, u) for C rows
    ent_u = np.concatenate([ks, signodes])
    ent_kind = np.concatenate([np.zeros(len(ks), np.int64),
                               np.ones(len(signodes), np.int64)])
    ent_g = np.concatenate([kd, signodes])
    ent_w = np.concatenate([-w2 * c_np[kd], -w2 * c_np[signodes]])

    owner = ent_u // R
    tl_loc = (ent_u - owner * R) // P
    col = (ent_u - owner * R) % P

    # |f0-1| below 1e-8 contributes <4e-8 abs output error — ignore, like
    # the tau edge pruning (consistent error budget).
    affected = np.unique(np.concatenate(
        [ent_u, np.nonzero(np.abs(f0_full - 1.0) > 1e-8)[0]]))

    # active local tiles per core
    act_tiles = [sorted(set(
        int((u - c * R) // P) for u in affected if u // R == c))
        for c in range(n_cores)]
    NT3 = max(1, max(len(a) for a in act_tiles))
    for a in act_tiles:
        pass  # pad handled below

    # per-core, per-active-tile edge lists
    core_data = []
    maxblk_per_slot = np.zeros(NT3, dtype=np.int64)
    for c in range(n_cores):
        tiles = act_tiles[c]
        percore = []
        for i in range(NT3):
            if i < len(tiles):
                t = tiles[i]
                m = (owner == c) & (tl_loc == t)
                percore.append((t, ent_g[m], ent_kind[m], ent_w[m], col[m]))
                maxblk_per_slot[i] = max(maxblk_per_slot[i],
                                         cdiv(len(ent_w[m]), P))
            else:
                percore.append((None, [], [], [], []))
        core_data.append(percore)

    NB3_t = maxblk_per_slot
    NBLK3 = int(NB3_t.sum())
    S3 = max(16, NBLK3 * P)

    # pack gather sources + idx per core
    z_b = bf(z_np)
    C_b = bf(C_np)
    maps = []
    M3 = 1
    packs = []
    for c in range(n_cores):
        keys = {}
        rows = []
        for (t, gg, kk, ww, cc) in core_data[c]:
            for gi, ki in zip(gg, kk):
                key = (int(ki), int(gi))
                if key not in keys:
                    keys[key] = len(rows)
                    rows.append(key)
        packs.append((keys, rows))
        M3 = max(M3, len(rows))
    M3 = max(16, M3)

    for c in range(n_cores):
        keys, rows = packs[c]
        zcarr = np.zeros((M3, D), dtype=BF)
        for r, (ki, gi) in enumerate(rows):
            zcarr[r] = C_b[gi] if ki else z_b[gi]
        idx = np.zeros(S3, dtype=np.int16)
        colid = np.full((max(NBLK3, 1), P), -1.0, dtype=np.float32)
        cslv = np.zeros((max(NBLK3, 1), P), dtype=np.float32)
        zogrid = np.zeros((P, NT3 * D), dtype=np.float32)
        f0grid = np.ones((P, NT3), dtype=np.float32)
        blk0 = 0
        tiles_used = []
        for i, (t, gg, kk, ww, cc) in enumerate(core_data[c]):
            nb = int(NB3_t[i])
            if t is not None:
                lo = c * prep["R"] + t * P
                nrow = min(P, prep["R"] - t * P)
                zogrid[:nrow, i * D : (i + 1) * D] = z_np[lo : lo + nrow]
                f0grid[:nrow, i] = f0_full[lo : lo + nrow]
                tiles_used.append((i, t, nrow))
                for e in range(len(ww)):
                    slot = blk0 * P + e
                    idx[slot] = keys[(int(kk[e]), int(gg[e]))]
                    colid[blk0 + e // P, e % P] = float(cc[e])
                    cslv[blk0 + e // P, e % P] = float(ww[e])
            blk0 += nb
        maps.append(dict(
            zc=zcarr,
            idx3=PassLayout._wrap(idx),
            colid=np.ascontiguousarray(colid.T),
            csl=np.ascontiguousarray(cslv.T),
            zog=zogrid,
            f0g=f0grid,
            _tiles=tiles_used,
        ))
    return dict(NT3=NT3, NB3_t=NB3_t, M3=M3, S3=S3, maps=maps)


# ----------------------------------------------------------------------------
# Runtime driver
# ----------------------------------------------------------------------------

def full_pipeline(inputs_np, runner, n_cores=N_CORES):
    N = inputs_np["x"].shape[0]
    prep = host_prep(inputs_np["edge_index"], N, n_cores)
    R = prep["R"]
    lay = prep["L12"]
    NT, RP = lay.NT, lay.RP
    src, dst = prep["src"], prep["dst"]
    dinv, indeg = prep["dinv"], prep["indeg"]
    gc = prep["gc"]
    if gc is None:
        gc = np.concatenate([np.arange(R, dtype=np.int64) for _ in range(n_cores)])

    Tv = float(np.asarray(inputs_np["temperature"]).reshape(-1)[0])
    wv = float(np.asarray(inputs_np["weight"]).reshape(-1)[0])
    w2 = 2.0 * wv
    b_np = np.asarray(inputs_np["b"], dtype=np.float64)

    # host: prescale x rows by dinv (src factor)
    xs = np.asarray(inputs_np["x"], dtype=np.float64) * dinv[:, None]
    xs_b = bf(xs)
    W_b = bf(inputs_np["W"])

    # L1
    nc1 = build_L1(lay, N)
    maps1 = []
    for c in range(n_cores):
        lc = lay.cores[c]
        lo = c * R
        xsT = np.zeros((P, RP), dtype=BF)
        xsT[:, gc[lo : lo + R]] = xs_b[lo : lo + R].T
        maps1.append(dict(
            x=xs_b, W=W_b,
            idxA=lc["idxA"], idxB=lc["idxB"],
            colid=bf(lc["colid"]), xsT=xsT,
        ))
    res1 = runner(nc1, maps1, ["zT"])

    # host: z = dinv * z~ + b ; q ; segq
    zt = np.concatenate(
        [np.asarray(res1[c]["zT"], dtype=np.float64)[:, gc[c * R:(c + 1) * R]].T
         for c in range(n_cores)], axis=0)  # [N, D]
    z_np = zt * dinv[:, None] + b_np[None, :]
    z_b = bf(z_np)
    q = np.einsum("nd,nd->n", z_np, z_np)
    segq = np.bincount(dst, weights=q[src], minlength=N)

    # L2
    nc2 = build_L2(lay, N)
    maps2 = []
    for c in range(n_cores):
        lc = lay.cores[c]
        maps2.append(dict(
            z=z_b,
            idxA=lc["idxA"], idxB=lc["idxB"],
            colid=bf(lc["colid"]),
        ))
    res2 = runner(nc2, maps2, ["CT"])

    C_np = np.concatenate(
        [np.asarray(res2[c]["CT"], dtype=np.float64)[:, gc[c * R:(c + 1) * R]].T
         for c in range(n_cores)], axis=0)  # [N, D]

    # host: E, softmax, c
    E = segq + indeg * q - 2.0 * np.einsum("nd,nd->n", z_np, C_np)
    a = -E / Tv
    m = a.max()
    ex = np.exp(a - m)
    S = ex.sum()
    p = ex / S
    logp = a - m - np.log(S)
    H = -np.sum(p * logp)
    c_np = (1.0 / Tv) * p * (logp + H)

    l3 = build_l3_layout(prep, z_np, C_np, c_np, w2)
    nc3 = build_L3(l3["NT3"], l3["NB3_t"], l3["M3"], l3["S3"])
    maps3 = [{k: v for k, v in m.items() if not k.startswith("_")}
             for m in l3["maps"]]
    res3 = runner(nc3, maps3, ["out"])

    out = np.ascontiguousarray(z_np.astype(np.float32))
    for c in range(n_cores):
        o = np.asarray(res3[c]["out"], dtype=np.float32)
        for (i, t, nrow) in l3["maps"][c]["_tiles"]:
            lo = c * R + t * P
            out[lo : lo + nrow] = o[:nrow, i * D : (i + 1) * D]
    return out


# ----------------------------------------------------------------------------
# Entry point + timing
# ----------------------------------------------------------------------------

TRACE = False
TIME_REPS = 0
LAST_EXEC_TIMES = []
STUB_TIMES = []
TRACE_TIMES = []


def _hw_runner(nc, in_maps, out_names):
    from concourse.bass_utils import run_bass_kernel_spmd
    res = run_bass_kernel_spmd(nc, in_maps, core_ids=list(range(len(in_maps))),
                               trace=TRACE)
    if TRACE and res.exec_time_ns:
        TRACE_TIMES.append(res.exec_time_ns)
    if TIME_REPS:
        k_ns, s_ns = _time_launch_pair(nc, _build_stub(nc), in_maps,
                                       max(3, TIME_REPS))
        LAST_EXEC_TIMES.append(k_ns)
        STUB_TIMES.append(s_ns)
    return res.results


KINNER = int(_os.environ.get("KINNER", "8"))


def _time_launch_pair(nc, nc_stub, in_maps, reps):
    """Interleave batches of kernel and stub executions so the ~80ms axon
    dispatch overhead (and its drift) cancels out of the difference; the
    inner batch of KINNER launches divides per-sample noise by KINNER."""
    import jax, time as _time
    fn_k, in_k, zo_k = _make_sharded(nc, in_maps)
    fn_s, in_s, zo_s = _make_sharded(nc_stub, in_maps)
    wk, ws = [], []
    for r in range(reps + 1):
        t0 = _time.perf_counter()
        for _ in range(KINNER):
            outs = fn_k(*in_k, *zo_k)
        jax.block_until_ready(outs)
        t1 = _time.perf_counter()
        for _ in range(KINNER):
            outs = fn_s(*in_s, *zo_s)
        jax.block_until_ready(outs)
        t2 = _time.perf_counter()
        if r:
            wk.append((t1 - t0) * 1e9 / KINNER)
            ws.append((t2 - t1) * 1e9 / KINNER)
    return min(wk), min(ws)


def run_sim(nc, in_maps, out_names):
    from concourse.bass_interp import MultiCoreSim
    n = len(in_maps)
    sim = MultiCoreSim(nc, num_cores=n)
    for c in range(n):
        for kk, v in in_maps[c].items():
            sim.cores[c].tensor(kk)[:] = v
    sim.simulate(check_with_hw=False)
    return [{kk: np.array(sim.cores[c].tensor(kk)) for kk in out_names}
            for c in range(n)]


def kernel(**inputs):
    inputs_np = {k: np.asarray(v) for k, v in inputs.items()}
    out = full_pipeline(inputs_np, _hw_runner)
    return out.astype(np.float32)


def _make_sharded(nc, in_maps):
    import jax
    import concourse.mybir as _mybir
    from concourse import bass2jax as b2j
    from jax.experimental.shard_map import shard_map
    from jax.sharding import Mesh, PartitionSpec, NamedSharding

    b2j.install_neuronx_cc_hook()
    n_cores = len(in_maps)
    partition_name = nc.partition_id_tensor.name if nc.partition_id_tensor else None
    in_names, out_names, out_avals, zero_outs = [], [], [], []
    for alloc in nc.m.functions[0].allocations:
        if not isinstance(alloc, _mybir.MemoryLocationSet):
            continue
        name = alloc.memorylocations[0].name
        if alloc.kind == "ExternalInput":
            if name != partition_name:
                in_names.append(name)
        elif alloc.kind == "ExternalOutput":
            shape = tuple(alloc.tensor_shape)
            dtype = _mybir.dt.np(alloc.dtype)
            out_names.append(name)
            out_avals.append(jax.core.ShapedArray(shape, dtype))
            zero_outs.append(np.zeros(shape, dtype))
    n_params = len(in_names)
    all_in = in_names + out_names
    if partition_name is not None:
        all_in = all_in + [partition_name]

    def _body(*args):
        operands = list(args)
        if partition_name is not None:
            operands.append(b2j.partition_id_tensor())
        outs = b2j._bass_exec_p.bind(
            *operands,
            out_avals=tuple(out_avals),
            in_names=tuple(all_in),
            out_names=tuple(out_names),
            lowering_input_output_aliases=(),
            sim_require_finite=True,
            sim_require_nnan=True,
            nc=nc,
        )
        return tuple(outs)

    devices = jax.devices()[:n_cores]
    mesh = Mesh(np.asarray(devices), ("core",))
    spec = PartitionSpec("core")
    in_specs = (spec,) * (n_params + len(out_names))
    out_specs = (spec,) * len(out_names)
    fn = jax.jit(
        shard_map(_body, mesh=mesh, in_specs=in_specs, out_specs=out_specs,
                  check_rep=False),
        keep_unused=True,
    )
    sh = NamedSharding(mesh, spec)
    concat_in = [
        jax.device_put(
            np.concatenate([np.asarray(in_maps[c][nm]) for c in range(n_cores)],
                           axis=0),
            sh,
        )
        for nm in in_names
    ]
    concat_zero = [
        jax.device_put(np.zeros((n_cores * z.shape[0], *z.shape[1:]), z.dtype), sh)
        for z in zero_outs
    ]
    return fn, concat_in, concat_zero


def _time_launch(nc, in_maps, reps):
    import jax, time as _time
    fn, concat_in, concat_zero = _make_sharded(nc, in_maps)
    walls = []
    for _ in range(reps + 1):
        t0 = _time.perf_counter()
        outs = fn(*concat_in, *concat_zero)
        jax.block_until_ready(outs)
        walls.append((_time.perf_counter() - t0) * 1e9)
    return min(walls[1:]) if len(walls) > 1 else walls[0]


def _build_stub(nc_ref):
    import concourse.mybir as _mybir
    nc = bacc.Bacc("TRN2", target_bir_lowering=False, debug=False)
    outs = []
    for alloc in nc_ref.m.functions[0].allocations:
        if not isinstance(alloc, _mybir.MemoryLocationSet):
            continue
        ml = alloc.memorylocations[0]
        if alloc.kind == "ExternalInput":
            if nc_ref.partition_id_tensor and ml.name == nc_ref.partition_id_tensor.name:
                continue
            nc.dram_tensor(ml.name, list(alloc.tensor_shape), alloc.dtype,
                           kind="ExternalInput")
        elif alloc.kind == "ExternalOutput":
            outs.append(nc.dram_tensor(ml.name, list(alloc.tensor_shape),
                                       alloc.dtype, kind="ExternalOutput"))
    with tile.TileContext(nc) as tc:
        with tc.tile_pool(name="w", bufs=1) as wp:
            for o in outs:
                t = wp.tile([1, 16], o.dtype, tag="t")
                nc.vector.memset(t[:], 0)
                sl = tuple([slice(0, 1)] * (len(o.shape) - 1)
                           + [slice(0, min(16, o.shape[-1]))])
                nc.sync.dma_start(o[sl], t[0:1, 0:min(16, o.shape[-1])])
    nc.compile()
    return nc
